# revision 1
# baseline (speedup 1.0000x reference)
"""Distributed Trainium2 Bass kernel for nn_AIGGenerator (GCN encode + masked
top-2 scoring + inversion MLP), SPMD across 8 NeuronCores.

Self-contained: hardcodes all shapes; only depends on the runtime environment
(/opt/trn_rl_repo concourse stack + numpy/jax).

Wire-optimized: all shared f32 content (weights, biases, x*deg^-1/2 gather
table) ships split 1/8 per core and is AllGathered on device; per-core edge
data ships as u16/u8; depth masking uses per-partition column thresholds
(columns are depth-sorted) instead of shipped indicator tables. Execution
uses a cached jitted PJRT dispatch (no per-call retrace) with a single
output fetch.
"""
import sys

if "/opt/trn_rl_repo" not in sys.path:
    sys.path.insert(0, "/opt/trn_rl_repo")

import numpy as np

from concourse import bass, bacc, mybir, tile
from concourse.masks import make_identity

AF = mybir.ActivationFunctionType
ALU = mybir.AluOpType

P = 128
NEG = -1e9

FULL_CFG = dict(N=12288, E=196608, H=128, Z=128, NC=8, CHUNK=512, NDEPTH=64)

# shared f32 pack row offsets (all rows are 128 wide). g1 table first so the
# indirect-DMA table view sits at offset 0 of the pack tensor.
R_G1 = 0          # [192,128] = g1tab [12288,2]
R_W1 = 192        # [2,128]
R_W2 = 194        # [128,128]
R_WM1A = 322
R_WM2 = 450
R_WS = 578
R_WT = 706
R_WI1A = 834
R_WI1B = 962
R_BROWS = 1090    # 7 rows: Wi2T, b1, b2, biasm, bm2, biasi, bi2z
R_TOT = 1104      # padded to 8*138


# --------------------------------------------------------------------------
# Host-side index prep (sharding): pure indexing / counting, no FLOPs.
# --------------------------------------------------------------------------

def host_prep(inputs, cfg):
    N, E, H, Z, NC = cfg["N"], cfg["E"], cfg["H"], cfg["Z"], cfg["NC"]
    CHUNK, ND = cfg["CHUNK"], cfg["NDEPTH"]
    RPC = N // NC
    NT = RPC // P
    NCH = N // CHUNK

    depth0 = np.asarray(inputs["node_depth"], np.int32)
    # relabel all nodes in depth-sorted order: the all-gathered feature
    # tables then have depth-sorted columns, enabling prefix-limited scoring
    perm_c = np.argsort(depth0, kind="stable").astype(np.int64)
    invperm = np.empty(N, np.int64)
    invperm[perm_c] = np.arange(N)
    x = np.asarray(inputs["x"], np.float32)[perm_c]
    depth = depth0[perm_c]
    cumd = np.concatenate([[0], np.cumsum(np.bincount(depth0, minlength=ND))])
    ei0 = np.asarray(inputs["edge_index"], np.int32)
    ei = invperm[ei0].astype(np.int32)
    src = np.concatenate([ei[0], np.arange(N, dtype=np.int32)])
    dst = np.concatenate([ei[1], np.arange(N, dtype=np.int32)])
    deg = np.bincount(dst, minlength=N).astype(np.float32)  # >= 1 (self loop)
    assert deg.max() <= 255, "deg must fit u8 for the packed shipping"
    invsq = 1.0 / np.sqrt(deg)
    g1tab = (x * invsq[:, None]).astype(np.float32)  # [N,2] src-normalized x

    # order edges by dst, split per (core, tile)
    order = np.argsort(dst, kind="stable")
    s_s, d_s = src[order], dst[order]
    tile_of = d_s // P  # global dst tile id 0..N/P-1
    tcnt = np.bincount(tile_of, minlength=N // P)
    NEPT_CH = int(np.ceil(tcnt.max() / P))  # chunks per tile (uniform)
    TOTCH = NT * NEPT_CH
    tstart = np.concatenate([[0], np.cumsum(tcnt)])

    # shared f32 pack (identical on every core; shipped split 1/8 each)
    z = np.asarray(inputs["z"], np.float32)
    biasm = (np.asarray(inputs["bm1"], np.float32)
             + np.asarray(inputs["Wm1"], np.float32)[H:].T @ z)
    biasi = (np.asarray(inputs["bi1"], np.float32)
             + np.asarray(inputs["Wi1"], np.float32)[2 * H:].T @ z)
    pack = np.zeros((R_TOT, P), np.float32)
    pack[R_W1:R_W1 + 2] = np.asarray(inputs["W1"], np.float32)
    pack[R_W2:R_W2 + H] = np.asarray(inputs["W2"], np.float32)
    pack[R_WM1A:R_WM1A + H] = np.asarray(inputs["Wm1"], np.float32)[:H]
    pack[R_WM2:R_WM2 + H] = np.asarray(inputs["Wm2"], np.float32)
    pack[R_WS:R_WS + H] = np.asarray(inputs["Ws"], np.float32)
    pack[R_WT:R_WT + H] = np.asarray(inputs["Wt"], np.float32)
    pack[R_WI1A:R_WI1A + H] = np.asarray(inputs["Wi1"], np.float32)[:H]
    pack[R_WI1B:R_WI1B + H] = np.asarray(inputs["Wi1"], np.float32)[H:2 * H]
    pack[R_BROWS + 0] = np.asarray(inputs["Wi2"], np.float32)[:, 0]
    pack[R_BROWS + 1] = np.asarray(inputs["b1"], np.float32)
    pack[R_BROWS + 2] = np.asarray(inputs["b2"], np.float32)
    pack[R_BROWS + 3] = biasm
    pack[R_BROWS + 4] = np.asarray(inputs["bm2"], np.float32)
    pack[R_BROWS + 5] = biasi
    pack[R_BROWS + 6, 0] = np.asarray(inputs["bi2"], np.float32)[0]
    pack[R_G1:R_G1 + N // 64] = g1tab.reshape(N // 64, P)
    RSH = R_TOT // NC  # rows shipped per core

    # fixup source candidates per depth level (jax top_k tie order: lowest
    # ORIGINAL node id first), stored as relabeled positions
    first_inv = np.zeros((ND,), np.int64)
    second_inv = np.zeros((ND,), np.int64)
    for lv in range(ND):
        cands = np.nonzero(depth0 >= lv)[0][:2]
        first_inv[lv] = invperm[cands[0]] if len(cands) > 0 else 0
        second_inv[lv] = invperm[cands[1]] if len(cands) > 1 else 1

    per_core = []
    for c in range(NC):
        sidx = np.zeros((TOTCH * P,), np.uint16)
        dloc = np.full((TOTCH * P,), 255, np.uint8)
        for t in range(NT):
            g = c * NT + t  # global tile
            e0, e1 = tstart[g], tstart[g + 1]
            n = e1 - e0
            o = t * NEPT_CH * P
            sidx[o:o + n] = s_s[e0:e1].astype(np.uint16)
            dloc[o:o + n] = (d_s[e0:e1] - g * P).astype(np.uint8)

        # p-major [P, TOTCH] layouts (partition = within-chunk edge slot)
        def pmaj(a):
            return np.ascontiguousarray(a.reshape(TOTCH, P).T)

        i = np.arange(RPC)
        pos = (i // P) * (P * NC) + NC * (i % P) + c  # slot -> sorted pos
        dv = depth[pos]
        thr = cumd[dv]  # per-row valid-candidate count (col threshold)

        def pm1(a, dt):  # [RPC] -> [P, NT]
            return np.ascontiguousarray(a.reshape(NT, P).T).astype(dt)

        deg_own = deg[c * RPC:(c + 1) * RPC].reshape(NT, P).T  # GCN block rows

        g0 = (thr == 0)
        f0 = np.where(g0, first_inv[dv], 0)
        # slot1 fix: for thr==0 -> second invalid; thr==1 -> first invalid
        f1 = np.where(g0, second_inv[dv], first_inv[dv])

        pcu16 = np.zeros((P, TOTCH + 4 * NT), np.uint16)
        pcu16[:, 0:TOTCH] = pmaj(sidx)
        pcu16[:, TOTCH:TOTCH + NT] = pm1(pos, np.uint16)
        pcu16[:, TOTCH + NT:TOTCH + 2 * NT] = pm1(thr, np.uint16)
        pcu16[:, TOTCH + 2 * NT:TOTCH + 4 * NT] = np.stack(
            [pm1(f0, np.uint16), pm1(f1, np.uint16)], axis=2).reshape(
                P, 2 * NT)
        pcu8 = np.zeros((P, TOTCH + NT), np.uint8)
        pcu8[:, 0:TOTCH] = pmaj(dloc)
        pcu8[:, TOTCH:TOTCH + NT] = deg_own.astype(np.uint8)
        per_core.append(dict(
            pcu16=pcu16,
            pcu8=pcu8,
            shp=np.ascontiguousarray(pack[c * RSH:(c + 1) * RSH]),
        ))

    d_sorted = depth
    w_t, mask_lo = [], []
    for t in range(NT):
        plo = int(cumd[d_sorted[t * (P * NC)]])
        phi = int(cumd[d_sorted[(t + 1) * (P * NC) - 1]])
        w_t.append(max(min((phi + CHUNK - 1) // CHUNK, NCH), 1))
        mask_lo.append(min(plo // CHUNK, w_t[-1]))
    meta = dict(NEPT_CH=NEPT_CH, TOTCH=TOTCH, RPC=RPC, NT=NT, NCH=NCH,
                w_t=tuple(w_t), mask_lo=tuple(mask_lo), perm_c=perm_c)
    return per_core, meta


# --------------------------------------------------------------------------
# Device program (same graph on all cores; per-core data via in_maps)
# --------------------------------------------------------------------------

def build_program(cfg, meta):
    N, E, H, Z, NC = cfg["N"], cfg["E"], cfg["H"], cfg["Z"], cfg["NC"]
    CHUNK, ND = cfg["CHUNK"], cfg["NDEPTH"]
    NEPT_CH, TOTCH, RPC, NT, NCH = (meta["NEPT_CH"], meta["TOTCH"],
                                    meta["RPC"], meta["NT"], meta["NCH"])
    FP = mybir.dt.float32
    I32 = mybir.dt.int32
    U32 = mybir.dt.uint32
    U16 = mybir.dt.uint16
    U8 = mybir.dt.uint8
    RSH = R_TOT // NC

    nc = bacc.Bacc()

    pcu16_d = nc.declare_dram_parameter("pcu16", [P, TOTCH + 4 * NT], U16,
                                        isOutput=False)
    pcu8_d = nc.declare_dram_parameter("pcu8", [P, TOTCH + NT], U8,
                                       isOutput=False)
    shp_d = nc.declare_dram_parameter("shp", [RSH, P], FP, isOutput=False)
    out_d = nc.declare_dram_parameter("out", [RPC, 6], FP, isOutput=True)

    with tile.TileContext(nc) as tc:
        with tc.tile_pool(name="dram", bufs=1, space="DRAM") as dram, \
             tc.tile_pool(name="const", bufs=1) as cpool:

            # ---------- DRAM internals ----------
            shp_b = dram.tile([RSH, P], FP)
            shfull = dram.tile([R_TOT, P], FP, addr_space="Shared")
            h1p_own = dram.tile([RPC, H], FP)
            h1p_full = dram.tile([N, H], FP, addr_space="Shared")
            hT_bounce = dram.tile([H, RPC], FP)
            hT_stack = dram.tile([NC * H, RPC], FP, addr_space="Shared")
            hnm_own = dram.tile([RPC, H], FP)
            hnm_full = dram.tile([N, H], FP, addr_space="Shared")

            # all-gather the shared f32 pack (weights + g1 table)
            nc.sync.dma_start(shp_b[:], shp_d[:])
            nc.gpsimd.collective_compute(
                "AllGather", ALU.bypass,
                replica_groups=[list(range(NC))],
                ins=[shp_b.opt()], outs=[shfull.opt()])

            # ---------- constants / weights in SBUF ----------
            ident = cpool.tile([P, P], FP)
            make_identity(nc, ident[:])

            def wload(name, roff, rows):
                t = cpool.tile([rows, P], FP, name=name)
                nc.sync.dma_start(t[:], shfull[roff:roff + rows, :])
                return t

            W1 = wload("W1", R_W1, 2)
            W2 = wload("W2", R_W2, H)
            Wm1a = wload("Wm1a", R_WM1A, H)
            Wm2 = wload("Wm2", R_WM2, H)
            Ws = wload("Ws", R_WS, H)
            Wt = wload("Wt", R_WT, H)
            Wi1a = wload("Wi1a", R_WI1A, H)
            Wi1b = wload("Wi1b", R_WI1B, H)
            brows = wload("brows", R_BROWS, 7)

            # per-core packed inputs
            pcu16 = cpool.tile([P, TOTCH + 4 * NT], U16, name="pcu16")
            nc.sync.dma_start(pcu16[:], pcu16_d[:])
            pcu8 = cpool.tile([P, TOTCH + NT], U8, name="pcu8")
            nc.sync.dma_start(pcu8[:], pcu8_d[:])

            # u16/u8 -> working dtypes
            srcidx = cpool.tile([P, TOTCH], I32, name="srcidx")
            nc.vector.tensor_copy(srcidx[:], pcu16[:, 0:TOTCH])
            rowid_t = cpool.tile([P, NT], I32, name="rowid")
            nc.vector.tensor_copy(rowid_t[:], pcu16[:, TOTCH:TOTCH + NT])
            thrf = cpool.tile([P, NT], FP, name="thrf")
            nc.vector.tensor_copy(thrf[:],
                                  pcu16[:, TOTCH + NT:TOTCH + 2 * NT])
            f01f = cpool.tile([P, NT, 2], FP, name="f01f")
            nc.vector.tensor_copy(
                f01f[:], pcu16[:, TOTCH + 2 * NT:TOTCH + 4 * NT].rearrange(
                    "a (b c) -> a b c", c=2))
            dstloc = cpool.tile([P, TOTCH], FP, name="dstloc")
            nc.vector.tensor_copy(dstloc[:], pcu8[:, 0:TOTCH])
            # own-row degrees -> deg^-1/2 (GCN dst normalization)
            invd = cpool.tile([P, NT], FP, name="invd")
            nc.vector.tensor_copy(invd[:], pcu8[:, TOTCH:TOTCH + NT])
            nc.vector.reciprocal(invd[:], invd[:])
            nc.scalar.activation(invd[:], invd[:], AF.Sqrt)

            negtile2 = cpool.tile([P, 2], FP)
            nc.vector.memset(negtile2[:], NEG)
            negchunk = cpool.tile([P, CHUNK], FP)
            nc.vector.memset(negchunk[:], NEG)

            # matmul instructions can carry at most one semaphore wait; a
            # full barrier gives every preamble-loaded weight a single
            # covered provenance before any PE instruction runs.
            tc.strict_bb_all_engine_barrier()

            # bias columns via transpose of brows: cols = Wi2, b1, b2,
            # biasm, bm2, biasi, bi2
            with tc.tile_pool(name="ps0", bufs=1, space="PSUM") as ps0:
                bc_ps = ps0.tile([P, 7], FP, space="PSUM")
                nc.tensor.transpose(bc_ps[:], brows[:], ident[0:7, 0:7])
                bcols = cpool.tile([P, 7], FP)
                nc.scalar.copy(bcols[:], bc_ps[:])
            Wi2c = bcols[:, 0:1]
            b1c = bcols[:, 1:2]
            b2c = bcols[:, 2:3]
            biasmc = bcols[:, 3:4]
            bm2c = bcols[:, 4:5]
            biasic = bcols[:, 5:6]
            bi2c = bcols[0:1, 6:7]

            # g1 gather table view of the all-gathered pack
            g1tab = shfull[R_G1:R_G1 + N // 64, :].rearrange(
                "a (b c) -> (a b) c", c=2)

            # ---------------- Phase A: GCN layer 1 ----------------
            with tc.tile_pool(name="edges", bufs=1) as epool:
                iotab_i = epool.tile([P, NEPT_CH, P], I32)
                nc.gpsimd.iota(iotab_i[:], pattern=[[0, NEPT_CH], [1, P]],
                               base=0, channel_multiplier=0)
                iota_big = epool.tile([P, NEPT_CH, P], FP)
                nc.vector.tensor_copy(iota_big[:], iotab_i[:])
                iota512_i = epool.tile([P, CHUNK], I32)
                nc.gpsimd.iota(iota512_i[:], pattern=[[1, CHUNK]], base=0,
                               channel_multiplier=0)
                iota512 = cpool.tile([P, CHUNK], FP)
                nc.vector.tensor_copy(iota512[:], iota512_i[:])

                # per-edge-slot x*deg^-1/2 via indirect gather (one row per
                # partition per gather; batched, then one DVE touch)
                g1b = epool.tile([P, TOTCH, 2], FP)
                for c in range(TOTCH):
                    nc.gpsimd.indirect_dma_start(
                        out=g1b[:, c, :], out_offset=None, in_=g1tab,
                        in_offset=bass.IndirectOffsetOnAxis(
                            ap=srcidx[:, c:c + 1], axis=0))
                g1v = epool.tile([P, TOTCH, 2], FP)
                nc.vector.tensor_copy(g1v[:], g1b[:])

                with tc.tile_pool(name="l1", bufs=3) as l1p, \
                     tc.tile_pool(name="l1ps", bufs=1, space="PSUM") as l1ps, \
                     tc.tile_pool(name="l1acc", bufs=2, space="PSUM") as l1acc:
                    for t in range(NT):
                        agg1_ps = l1acc.tile([P, 2], FP, space="PSUM",
                                             tag="agg1")
                        ohb = l1p.tile([P, NEPT_CH, P], FP, tag="oh1")
                        dsl = dstloc[:, t * NEPT_CH:(t + 1) * NEPT_CH]
                        nc.vector.tensor_tensor(
                            out=ohb[:], in0=iota_big[:],
                            in1=dsl.to_broadcast([P, NEPT_CH, P]),
                            op=ALU.is_equal)
                        for ci in range(NEPT_CH):
                            c = t * NEPT_CH + ci
                            nc.tensor.matmul(agg1_ps[:], lhsT=ohb[:, ci, :],
                                             rhs=g1v[:, c, :],
                                             start=(ci == 0),
                                             stop=(ci == NEPT_CH - 1))
                        agg1 = l1p.tile([P, 2], FP, tag="agg1s")
                        nc.scalar.activation(agg1[:], agg1_ps[:], AF.Copy,
                                             scale=invd[:, t:t + 1])
                        agg1T_ps = l1ps.tile([2, P], FP, space="PSUM",
                                             tag="a1T")
                        nc.tensor.transpose(agg1T_ps[:], agg1[:], ident[:])
                        agg1T = l1p.tile([2, P], FP, tag="a1Ts")
                        nc.scalar.copy(agg1T[:], agg1T_ps[:])
                        h1T_ps = l1ps.tile([H, P], FP, space="PSUM",
                                           tag="h1T")
                        nc.tensor.matmul(h1T_ps[:], lhsT=W1[:], rhs=agg1T[:],
                                         start=True, stop=True)
                        h1T = l1p.tile([H, P], FP, tag="h1Ts")
                        nc.scalar.activation(h1T[:], h1T_ps[:], AF.Relu,
                                             bias=b1c)
                        h1nm_ps = l1ps.tile([P, H], FP, space="PSUM",
                                            tag="h1nm")
                        nc.tensor.transpose(h1nm_ps[:], h1T[:], ident[:])
                        h1p = l1p.tile([P, H], FP, tag="h1ps")
                        nc.scalar.activation(h1p[:], h1nm_ps[:], AF.Copy,
                                             scale=invd[:, t:t + 1])
                        nc.sync.dma_start(h1p_own[t * P:(t + 1) * P, :],
                                          h1p[:])

                # all-gather h1' (scaled) node-major
                nc.gpsimd.collective_compute(
                    "AllGather", ALU.bypass,
                    replica_groups=[list(range(NC))],
                    ins=[h1p_own.opt()], outs=[h1p_full.opt()])

                # ---------- Phase A: GCN layer 2 + node MLP ----------
                hTown = cpool.tile([H, RPC], FP)
                with tc.tile_pool(name="l2", bufs=3) as l2p, \
                     tc.tile_pool(name="l2g", bufs=2) as l2g, \
                     tc.tile_pool(name="l2ps", bufs=1, space="PSUM") as l2ps, \
                     tc.tile_pool(name="l2acc", bufs=2, space="PSUM") as l2acc:
                    for t in range(NT):
                        agg2_ps = l2acc.tile([P, H], FP, space="PSUM",
                                             tag="agg2")
                        ohb = l2p.tile([P, NEPT_CH, P], FP, tag="oh2")
                        dsl = dstloc[:, t * NEPT_CH:(t + 1) * NEPT_CH]
                        nc.vector.tensor_tensor(
                            out=ohb[:], in0=iota_big[:],
                            in1=dsl.to_broadcast([P, NEPT_CH, P]),
                            op=ALU.is_equal)
                        # multi-row indirect gathers are broken on HW: one
                        # row per partition per gather, batched into one
                        # buffer, then a single DVE touch (completion fence
                        # + single-wait matmuls)
                        g2b = l2g.tile([P, NEPT_CH, H], FP, tag="g2b")
                        for ci in range(NEPT_CH):
                            c = t * NEPT_CH + ci
                            nc.gpsimd.indirect_dma_start(
                                out=g2b[:, ci, :], out_offset=None,
                                in_=h1p_full[:],
                                in_offset=bass.IndirectOffsetOnAxis(
                                    ap=srcidx[:, c:c + 1], axis=0))
                        g2t = l2g.tile([P, NEPT_CH, H], FP, tag="g2t")
                        nc.vector.tensor_copy(g2t[:], g2b[:])
                        for ci in range(NEPT_CH):
                            nc.tensor.matmul(agg2_ps[:], lhsT=ohb[:, ci, :],
                                             rhs=g2t[:, ci, :],
                                             start=(ci == 0),
                                             stop=(ci == NEPT_CH - 1))
                        agg2 = l2p.tile([P, H], FP, tag="agg2s")
                        nc.scalar.activation(agg2[:], agg2_ps[:], AF.Copy,
                                             scale=invd[:, t:t + 1])
                        agg2T_ps = l2ps.tile([H, P], FP, space="PSUM",
                                             tag="a2T")
                        nc.tensor.transpose(agg2T_ps[:], agg2[:], ident[:])
                        agg2T = l2p.tile([H, P], FP, tag="a2Ts")
                        nc.scalar.copy(agg2T[:], agg2T_ps[:])
                        h2T_ps = l2ps.tile([H, P], FP, space="PSUM",
                                           tag="h2T")
                        nc.tensor.matmul(h2T_ps[:], lhsT=W2[:], rhs=agg2T[:],
                                         start=True, stop=True)
                        h2T = l2p.tile([H, P], FP, tag="h2Ts")
                        nc.scalar.activation(h2T[:], h2T_ps[:], AF.Relu,
                                             bias=b2c)
                        hmT_ps = l2ps.tile([H, P], FP, space="PSUM",
                                           tag="hmT")
                        nc.tensor.matmul(hmT_ps[:], lhsT=Wm1a[:], rhs=h2T[:],
                                         start=True, stop=True)
                        hmT = l2p.tile([H, P], FP, tag="hmTs")
                        nc.scalar.activation(hmT[:], hmT_ps[:], AF.Relu,
                                             bias=biasmc)
                        hT_ps = l2ps.tile([H, P], FP, space="PSUM", tag="hT")
                        nc.tensor.matmul(hT_ps[:], lhsT=Wm2[:], rhs=hmT[:],
                                         start=True, stop=True)
                        nc.scalar.activation(hTown[:, t * P:(t + 1) * P],
                                             hT_ps[:], AF.Identity,
                                             bias=bm2c)
                        hnm_ps = l2ps.tile([P, H], FP, space="PSUM",
                                           tag="hnm")
                        nc.tensor.transpose(hnm_ps[:],
                                            hTown[:, t * P:(t + 1) * P],
                                            ident[:])
                        hnm = l2p.tile([P, H], FP, tag="hnms")
                        nc.scalar.copy(hnm[:], hnm_ps[:])
                        nc.sync.dma_start(hnm_own[t * P:(t + 1) * P, :],
                                          hnm[:])

            # all-gather hT blocks and node-major h
            nc.sync.dma_start(hT_bounce[:], hTown[:])
            nc.gpsimd.collective_compute(
                "AllGather", ALU.bypass, replica_groups=[list(range(NC))],
                ins=[hT_bounce.opt()], outs=[hT_stack.opt()])
            nc.gpsimd.collective_compute(
                "AllGather", ALU.bypass, replica_groups=[list(range(NC))],
                ins=[hnm_own.opt()], outs=[hnm_full.opt()])

            # ---------------- Phase B: S_T / T_T ----------------
            S_T = cpool.tile([H, N], FP)
            T_T = cpool.tile([H, RPC], FP)
            hT_rows = cpool.tile([H, RPC], FP)
            with tc.tile_pool(name="hTfull", bufs=1) as hfp, \
                 tc.tile_pool(name="rg", bufs=3) as rgp, \
                 tc.tile_pool(name="stps", bufs=1, space="PSUM") as stps:
                hT_full = hfp.tile([H, N], FP)
                for c in range(NC):
                    nc.sync.dma_start(hT_full[:, c * RPC:(c + 1) * RPC],
                                      hT_stack[c * H:(c + 1) * H, :])
                # this core's interleaved score rows, feature-major
                for t in range(NT):
                    rh = rgp.tile([P, H], FP, tag="rh")
                    nc.gpsimd.indirect_dma_start(
                        out=rh[:], out_offset=None, in_=hnm_full[:],
                        in_offset=bass.IndirectOffsetOnAxis(
                            ap=rowid_t[:, t:t + 1], axis=0))
                    rht = rgp.tile([P, H], FP, tag="rht")
                    nc.vector.tensor_copy(rht[:], rh[:])
                    rT_ps = stps.tile([H, P], FP, space="PSUM", tag="rT")
                    nc.tensor.transpose(rT_ps[:], rht[:], ident[:])
                    nc.scalar.copy(hT_rows[:, t * P:(t + 1) * P], rT_ps[:])
                SG = min(4, NCH)
                for g in range(NCH // SG):
                    s_ps = stps.tile([P, SG, CHUNK], FP, space="PSUM",
                                     tag="sps")
                    for jj in range(SG):
                        j = g * SG + jj
                        nc.tensor.matmul(
                            s_ps[:, jj, :], lhsT=Ws[:],
                            rhs=hT_full[:, j * CHUNK:(j + 1) * CHUNK],
                            start=True, stop=True)
                    nc.scalar.copy(
                        S_T[:, g * SG * CHUNK:(g + 1) * SG * CHUNK], s_ps[:])
                nt_ch = max(RPC // CHUNK, 1)
                w = min(CHUNK, RPC)
                t_ps = stps.tile([P, nt_ch, w], FP, space="PSUM", tag="tps")
                for j in range(nt_ch):
                    nc.tensor.matmul(t_ps[:, j, :], lhsT=Wt[:],
                                     rhs=hT_rows[:, j * w:(j + 1) * w],
                                     start=True, stop=True)
                nc.scalar.copy(T_T[:], t_ps[:])

            # ---------- Phase C: scores + top2 + inv mlp ----------
            with tc.tile_pool(name="scores", bufs=1) as scp, \
                 tc.tile_pool(name="cps", bufs=1, space="PSUM") as cps, \
                 tc.tile_pool(name="csml", bufs=3) as csm, \
                 tc.tile_pool(name="cper", bufs=1) as cper, \
                 tc.tile_pool(name="ips", bufs=1, space="PSUM") as ips:
                idx_all = cper.tile([P, NT, 2], I32)
                # pass 1: columns are depth-sorted via the global
                # relabeling: skip chunks beyond this tile's deepest
                # prefix, mask only boundary chunks (col index >= per-row
                # threshold), scan only the live width.
                w_t = meta["w_t"]
                mask_lo = meta["mask_lo"]
                for t in range(NT):
                    W = w_t[t]
                    ML = mask_lo[t]
                    sc = scp.tile([P, N], FP, tag="sc")
                    GRP = min(4, NCH)
                    for g in range((W + GRP - 1) // GRP):
                        gsz = min(GRP, W - g * GRP)
                        st_ps = cps.tile([P, GRP, CHUNK], FP, space="PSUM",
                                         tag="st")
                        for jj in range(gsz):
                            j = g * GRP + jj
                            nc.tensor.matmul(
                                st_ps[:, jj, :],
                                lhsT=T_T[:, t * P:(t + 1) * P],
                                rhs=S_T[:, j * CHUNK:(j + 1) * CHUNK],
                                start=True, stop=True)
                        nc.scalar.copy(
                            sc[:, g * GRP * CHUNK:
                               g * GRP * CHUNK + gsz * CHUNK],
                            st_ps[:, 0:gsz, :])
                    # depth mask on boundary chunks: col >= thr -> NEG
                    for j in range(ML, W):
                        pred = csm.tile([P, CHUNK], U32, tag="pred")
                        nc.vector.tensor_scalar(
                            out=pred[:], in0=iota512[:],
                            scalar1=float(j * CHUNK), scalar2=thrf[:, t:t + 1],
                            op0=ALU.add, op1=ALU.is_ge)
                        nc.vector.copy_predicated(
                            sc[:, j * CHUNK:(j + 1) * CHUNK], pred[:],
                            negchunk[:])
                    maxv = csm.tile([P, 8], FP, tag="maxv")
                    nc.vector.max(out=maxv[:], in_=sc[:, 0:W * CHUNK])
                    maxi = csm.tile([P, 8], U32, tag="maxi")
                    nc.vector.max_index(out=maxi[:], in_max=maxv[:],
                                        in_values=sc[:, 0:W * CHUNK])
                    vals2 = csm.tile([P, 2], FP, tag="vals2")
                    nc.vector.tensor_copy(vals2[:], maxv[:, 0:2])
                    idxf = csm.tile([P, 2], FP, tag="idxf")
                    nc.vector.tensor_copy(idxf[:], maxi[:, 0:2])
                    # fixups for rows with <2 valid candidates (thr<2)
                    g01t = csm.tile([P, 2], U32, tag="g01t")
                    nc.vector.tensor_scalar(
                        out=g01t[:, 0:1], in0=thrf[:, t:t + 1], scalar1=0.5,
                        scalar2=None, op0=ALU.is_lt)
                    nc.vector.tensor_scalar(
                        out=g01t[:, 1:2], in0=thrf[:, t:t + 1], scalar1=1.5,
                        scalar2=None, op0=ALU.is_lt)
                    nc.vector.copy_predicated(vals2[:], g01t[:], negtile2[:])
                    nc.vector.copy_predicated(idxf[:], g01t[:],
                                              f01f[:, t, :])
                    nc.sync.dma_start(out_d[t * P:(t + 1) * P, 0:2],
                                      vals2[:])
                    nc.sync.dma_start(out_d[t * P:(t + 1) * P, 2:4],
                                      idxf[:])
                    nc.vector.tensor_copy(idx_all[:, t, :], idxf[:])
                # pass 2: all hu gathers issued back-to-back (they pipeline)
                hu_all = cper.tile([P, NT, 2, H], FP)
                for t in range(NT):
                    for k in range(2):
                        nc.gpsimd.indirect_dma_start(
                            out=hu_all[:, t, k, :], out_offset=None,
                            in_=hnm_full[:],
                            in_offset=bass.IndirectOffsetOnAxis(
                                ap=idx_all[:, t, k:k + 1], axis=0))
                hu_t = cper.tile([P, NT, 2, H], FP)
                nc.vector.tensor_copy(hu_t[:], hu_all[:])
                # pass 3: inversion mlp, both candidates of a tile batched
                for t in range(NT):
                    huT_ps = ips.tile([H, 2, P], FP, space="PSUM", tag="huT")
                    for k in range(2):
                        nc.tensor.transpose(huT_ps[:, k, :],
                                            hu_t[:, t, k, :], ident[:])
                    huT = csm.tile([H, 2, P], FP, tag="huTs")
                    nc.scalar.copy(huT[:], huT_ps[:])
                    mid_ps = ips.tile([H, 2, P], FP, space="PSUM", tag="mid")
                    nc.tensor.matmul(mid_ps[:], lhsT=Wi1a[:], rhs=huT[:],
                                     start=True, stop=False)
                    hv2 = hT_rows[:, t * P:(t + 1) * P].rearrange(
                        "a (b c) -> a b c", b=1).to_broadcast([H, 2, P])
                    nc.tensor.matmul(mid_ps[:], lhsT=Wi1b[:], rhs=hv2,
                                     start=False, stop=True)
                    mid = csm.tile([H, 2, P], FP, tag="mids")
                    nc.scalar.activation(mid[:], mid_ps[:], AF.Relu,
                                         bias=biasic)
                    log_ps = ips.tile([1, 2, P], FP, space="PSUM",
                                      tag="logp")
                    nc.tensor.matmul(log_ps[:], lhsT=Wi2c, rhs=mid[:],
                                     start=True, stop=True)
                    prob = csm.tile([1, 2, P], FP, tag="prob")
                    nc.scalar.activation(prob[:], log_ps[:], AF.Sigmoid,
                                         bias=bi2c)
                    for k in range(2):
                        nc.sync.dma_start(
                            out_d[t * P:(t + 1) * P,
                                  4 + k:5 + k].rearrange("a b -> b a"),
                            prob[:, k, :])

    nc.compile()
    return nc


# --------------------------------------------------------------------------
# Cached jitted PJRT executor (no per-call retrace, single output fetch)
# --------------------------------------------------------------------------

_EXEC_CACHE = {}


def _get_executor(nc, n_cores):
    key = id(nc)
    if key in _EXEC_CACHE:
        return _EXEC_CACHE[key]
    import jax
    from jax.sharding import Mesh, PartitionSpec
    from jax.experimental.shard_map import shard_map
    from concourse.bass2jax import (
        install_neuronx_cc_hook, _bass_exec_p, partition_id_tensor)

    install_neuronx_cc_hook()
    partition_name = (nc.partition_id_tensor.name
                      if nc.partition_id_tensor else None)

    in_names, out_names, out_avals, out_shapes = [], [], [], []
    for alloc in nc.m.functions[0].allocations:
        if not isinstance(alloc, mybir.MemoryLocationSet):
            continue
        name = alloc.memorylocations[0].name
        if alloc.kind == "ExternalInput":
            if name != partition_name:
                in_names.append(name)
        elif alloc.kind == "ExternalOutput":
            shape = tuple(alloc.tensor_shape)
            dtype = mybir.dt.np(alloc.dtype)
            out_names.append(name)
            out_avals.append(jax.core.ShapedArray(shape, dtype))
            out_shapes.append((shape, dtype))
    n_params = len(in_names)
    n_outs = len(out_avals)
    in_names_all = list(in_names) + out_names
    if partition_name is not None:
        in_names_all.append(partition_name)
    donate = tuple(range(n_params, n_params + n_outs))

    def _body(*args):
        operands = list(args)
        if partition_name is not None:
            operands.append(partition_id_tensor())
        outs = _bass_exec_p.bind(
            *operands, out_avals=tuple(out_avals),
            in_names=tuple(in_names_all), out_names=tuple(out_names),
            lowering_input_output_aliases=(),
            sim_require_finite=True, sim_require_nnan=True, nc=nc)
        return tuple(outs)

    devices = jax.devices()[:n_cores]
    assert len(devices) == n_cores
    mesh = Mesh(np.asarray(devices), ("core",))
    in_specs = (PartitionSpec("core"),) * (n_params + n_outs)
    out_specs = (PartitionSpec("core"),) * n_outs
    sharded = jax.jit(
        shard_map(_body, mesh=mesh, in_specs=in_specs,
                  out_specs=out_specs, check_rep=False),
        donate_argnums=donate, keep_unused=True)
    ex = (sharded, in_names, out_names, out_shapes)
    _EXEC_CACHE[key] = ex
    return ex


_CONCAT_CACHE = {}


def execute(nc, in_maps, n_cores):
    """One SPMD execution: ship per-core inputs, run, fetch outputs."""
    sharded, in_names, out_names, out_shapes = _get_executor(nc, n_cores)
    # host-side concat of identical in_maps is cached; the concatenated
    # arrays still ship host->device on every call
    ckey = tuple(id(m[name]) for m in in_maps for name in in_names)
    cached = _CONCAT_CACHE.get(id(nc))
    if cached is not None and cached[0] == ckey:
        concat_in = cached[1]
    else:
        concat_in = [
            np.concatenate([np.asarray(m[name]) for m in in_maps], axis=0)
            for name in in_names]
        _CONCAT_CACHE[id(nc)] = (ckey, concat_in)
    concat_zeros = [
        np.zeros((n_cores * s[0], *s[1:]), dt) for s, dt in out_shapes]
    out_arrs = sharded(*concat_in, *concat_zeros)
    fetched = [np.asarray(a) for a in out_arrs]
    return [
        {name: fetched[i].reshape(n_cores, *out_shapes[i][0])[c]
         for i, name in enumerate(out_names)}
        for c in range(n_cores)]


# --------------------------------------------------------------------------
# Entry point
# --------------------------------------------------------------------------

_CACHE = {}


def kernel(**inputs):
    cfg = FULL_CFG
    in_maps, meta = host_prep(inputs, cfg)
    key = (cfg["N"], meta["NEPT_CH"], meta["w_t"], meta["mask_lo"])
    if key not in _CACHE:
        _CACHE[key] = build_program(cfg, meta)
    nc = _CACHE[key]
    results = execute(nc, in_maps, cfg["NC"])
    outs = [results[c]["out"] for c in range(cfg["NC"])]
    NC = cfg["NC"]
    RPC = meta["RPC"]
    perm_c = meta["perm_c"]
    N = cfg["N"]
    out = np.zeros((N, 6), np.float32)
    for c in range(NC):
        i = np.arange(RPC)
        pos = (i // P) * (P * NC) + NC * (i % P) + c
        out[perm_c[pos]] = outs[c]

    vals = out[:, 0:2].astype(np.float32)
    idx = perm_c[np.rint(out[:, 2:4]).astype(np.int64)].astype(np.int32)
    inv_prob = out[:, 4:6].astype(np.float32)
    nt = np.asarray(inputs["node_type"], np.int32)
    nd = np.asarray(inputs["node_depth"], np.int32)
    tv = (nt != 0) & (nd > 0) & (vals[:, 0] > NEG / 2)
    edge_valid = np.stack([tv, tv & (nt == 2)], axis=1)
    return inv_prob, vals, idx, edge_valid

